# revision 1
# baseline (speedup 1.0000x reference)
"""Multi-head attention forward on 8 TRN2 NeuronCores, data-parallel over batch.

Reference computation (per batch element b):
    qkv  = x @ qkv_w.T + qkv_b                     # [N, 3D]
    q, k = LN_headdim(q), LN_headdim(k)            # layernorm over head_dim=64
    S    = q @ k.T * hd^-0.5 ; A = softmax_j(S)    # per head
    out  = (A @ v) @ proj_w.T + proj_b             # [N, D]

Design (one batch element per core, no collectives; measured 216us/NEFF):
  - Host-side layout prep: x and weight matrices are transposed + bf16-cast on
    the CPU so the kernel DMAs them straight into contraction-on-partitions
    SBUF layouts (no on-device transposes or casts for the big operands).
  - qkv bias is applied BY TensorE: a rank-1 (ones x bias_row) matmul is the
    7th accumulation of each qkv chunk, so PSUM evacuation is a pure ScalarE
    copy and VectorE stays free for the LN reductions.
  - LayerNorm: one fused reduce per chunk produces sums AND sum-of-squares
    (the square is written next to the data so a single [128,16,64] reduce
    covers both); stats math batched per token-tile.  LN runs in two sweeps
    (all q + first k pairs first) so the attention phase starts while the
    remaining k chunks still compute.
  - Scores computed TRANSPOSED: ST[j,i] = k_j . q_i so E = exp(ST*scale) lands
    with contraction axis j on partitions -- E is directly the rhs/operand of
    the attn@v matmul with V as lhsT (no attention-matrix transpose).
  - K=128 scores via zero-padding: lhsT is the packed head-PAIR kkT, rhs is
    per-head q zero-padded in the other head's partitions.  Cross-head
    products multiply zeros; the full-height matmul keeps the PE HAM
    clock-gate warm (half-height K=64 matmuls leave the PE at 1.2 GHz).
  - Softmax denominators are free: V carries 64 ones-columns, so attn@v PSUM
    rows 64:128 hold s[i] already broadcast across 64 partitions; normalize is
    a fast approximate reciprocal + one multiply writing attnoutT directly.
  - No max-subtraction in softmax: q,k are layernormed so |q.k|*scale <= 8 and
    exp() is safely bounded in f32/bf16.
  - Heads are software-pipelined: head h's score matmuls interleave 1:1 with
    head h-1's attn@v accumulation so TensorE always has ready work while
    ScalarE drains exps (ScalarE is the attention-phase bottleneck at ~100%).
  - q/k head-pair transposes ride the otherwise-idle DMA engines: qn/kn are
    stored pair-contiguous so one dma_start_transpose per tensor per pair
    block-transposes all eight 128x128 tiles.
  - Projection computes outT = projwT.T @ attnoutT (e on partitions, bias via
    per-partition ScalarE activation); the host flips the [D,N] result back.
"""

import sys

import numpy as np

sys.path.insert(0, "/opt/trn_rl_repo")

from contextlib import ExitStack

import concourse.bass as bass
import concourse.tile as tile
from concourse import bacc, mybir
from concourse.bass_utils import run_bass_kernel_spmd

B, N, D = 8, 1024, 768
H, HD = 12, 64
O3 = 3 * D  # 2304
P = 128
NT = N // P  # 8 token tiles
DC = D // P  # 6 contraction subtiles
EPS = 1e-5
SCALE = HD ** -0.5  # 0.125
F32 = mybir.dt.float32
BF16 = mybir.dt.bfloat16

# qkv output chunks: [start, size]; q = o[0:768), k = [768:1536), v = [1536:2304)
QKV_CHUNKS = [(0, 512), (512, 512), (1024, 512), (1536, 512), (2048, 256)]


def _bcast_ap(ap_1d, parts):
    """View a 1-D DRAM AP as [parts, n] with partition stride 0 (broadcast)."""
    return bass.AP(
        tensor=ap_1d.tensor,
        offset=ap_1d.offset,
        ap=[[0, parts]] + list(ap_1d.ap),
    )


def _build_graph(apply_gn):
    nc = bacc.Bacc("TRN2", target_bir_lowering=False, debug=False, num_devices=B)

    # x and the weight matrices arrive pre-transposed and pre-cast to bf16
    # (host-side layout prep in kernel()): contraction dim on partitions.
    x_d = nc.dram_tensor("x", [D, N], BF16, kind="ExternalInput").ap()
    qkvw_d = nc.dram_tensor("qkv_w", [D, O3], BF16, kind="ExternalInput").ap()
    qkvb_d = nc.dram_tensor("qkv_b", [O3], F32, kind="ExternalInput").ap()
    qkvbr_d = nc.dram_tensor("qkv_br", [1, O3], BF16, kind="ExternalInput").ap()
    projw_d = nc.dram_tensor("proj_w", [D, D], BF16, kind="ExternalInput").ap()
    projb_d = nc.dram_tensor("proj_b", [D], F32, kind="ExternalInput").ap()
    gamma_d = nc.dram_tensor("qn_gamma", [HD], F32, kind="ExternalInput").ap()
    beta_d = nc.dram_tensor("qn_beta", [HD], F32, kind="ExternalInput").ap()
    # output is produced TRANSPOSED ([e, t]); the host flips it back
    out_d = nc.dram_tensor("out", [D, N], F32, kind="ExternalOutput").ap()

    with tile.TileContext(nc) as tc:
        _emit(tc, out_d, x_d, qkvw_d, qkvb_d, qkvbr_d, projw_d, projb_d,
              gamma_d, beta_d, apply_gn)

    nc.compile()
    return nc


def _emit(tc, out_d, x_d, qkvw_d, qkvb_d, qkvbr_d, projw_d, projb_d,
          gamma_d, beta_d, apply_gn):
    nc = tc.nc
    ctx = ExitStack()
    with ctx:
        const = ctx.enter_context(tc.tile_pool(name="const", bufs=1))
        wpool = ctx.enter_context(tc.tile_pool(name="wts", bufs=1))
        data = ctx.enter_context(tc.tile_pool(name="data", bufs=1))
        epool = ctx.enter_context(tc.tile_pool(name="escore", bufs=2))
        qkpool = ctx.enter_context(tc.tile_pool(name="qk", bufs=2))
        tmpp = ctx.enter_context(tc.tile_pool(name="tmp", bufs=5))
        stat = ctx.enter_context(tc.tile_pool(name="stat", bufs=4))
        outp = ctx.enter_context(tc.tile_pool(name="outp", bufs=3))
        nrm = ctx.enter_context(tc.tile_pool(name="nrm", bufs=2))

        # ---- constants ----
        brow = const.tile([1, O3], BF16)
        nc.sync.dma_start(brow[:], qkvbr_d[:])
        ones1 = const.tile([1, P], BF16)
        nc.vector.memset(ones1[:], 1.0)
        projb_col = const.tile([P, DC], F32)
        nc.sync.dma_start(projb_col[:], projb_d.rearrange("(et p) -> p et", p=P))
        eps_t = const.tile([P, 1], F32)
        nc.vector.memset(eps_t[:], EPS)
        if apply_gn:
            gamma_bc = const.tile([P, HD], F32)
            nc.sync.dma_start(gamma_bc[:], _bcast_ap(gamma_d, P))
            beta_bc = const.tile([P, HD], F32)
            nc.sync.dma_start(beta_bc[:], _bcast_ap(beta_d, P))

        # ---- DMA x and weights straight into [k, ., m] SBUF layouts ----
        xT = wpool.tile([P, DC, N], BF16)      # [d_in, d_out, t]
        qkvwT = wpool.tile([P, DC, O3], BF16)  # [d_in, d_out, o]
        projwT = wpool.tile([P, DC, D], BF16)  # [o_in, o_out, e]
        for dc in range(DC):
            nc.sync.dma_start(
                xT[:, dc, :],
                x_d.rearrange("(dc p) t -> p dc t", p=P)[:, dc, :],
            )
        for oc5 in range(0, O3, 512):
            for dc in range(DC):
                nc.sync.dma_start(
                    qkvwT[:, dc, oc5:min(oc5 + 512, O3)],
                    qkvw_d.rearrange("(dc p) o -> p dc o", p=P)[
                        :, dc, oc5:min(oc5 + 512, O3)],
                )
        for dc in range(DC):
            nc.sync.dma_start(
                projwT[:, dc, :],
                projw_d.rearrange("(dc p) e -> p dc e", p=P)[:, dc, :],
            )

        # q/k normalized, stored per head-pair: [t_in, pair, t_out, o_in_pair]
        # so one dma_start_transpose per pair yields qqT/kkT ([hd, t] blocks)
        qnp = data.tile([P, DC, NT, P], BF16)
        knp = data.tile([P, DC, NT, P], BF16)
        # v with 64 ones-columns: attn@v psum rows 64:128 become the softmax
        # denominator s[i], broadcast across 64 partitions by the PE for free
        vext = data.tile([P, NT, H, 2 * HD], BF16)
        nc.vector.memset(vext[:, :, :, HD:2 * HD], 1.0)

        with tc.tile_pool(name="ps_mm", bufs=5, space="PSUM") as ps_mm, \
             tc.tile_pool(name="ps_v", bufs=2, space="PSUM") as ps_v:
            # ---- QKV projection + bias + head-dim layernorm on q,k ----
            # Two sweeps: chunks {0,1} (all of q + k pairs 0-1) first so the
            # attention phase can start while chunk {2} (k pairs 2-5) runs.
            def emit_ln_sweep(chunks):
                nch = len(chunks)
                for tt in range(NT):
                    stats_tt = stat.tile([P, 48], F32, tag="stats",
                                         name="stats_tt")
                    tmps = {}
                    for i, ci in enumerate(chunks):
                        c0 = 512 * ci
                        psum_full = ps_mm.tile([P, 512], F32, tag="mm",
                                               name="psum_mm")
                        for dc in range(DC):
                            nc.tensor.matmul(
                                psum_full,
                                lhsT=xT[:, dc, tt * P:(tt + 1) * P],
                                rhs=qkvwT[:, dc, c0:c0 + 512],
                                start=(dc == 0),
                                stop=False,
                            )
                        nc.tensor.matmul(
                            psum_full, lhsT=ones1[:], rhs=brow[:, c0:c0 + 512],
                            start=False, stop=True,
                        )
                        tmp2 = tmpp.tile([P, 2, 512], F32, tag="tmp2",
                                         name="tmp2")
                        nc.scalar.copy(tmp2[:, 0, :], psum_full)
                        nc.scalar.square(tmp2[:, 1, :], psum_full)
                        nc.vector.tensor_reduce(
                            stats_tt[:, i * 16:(i + 1) * 16],
                            tmp2.rearrange("p b (s h) -> p (b s) h", h=HD),
                            axis=mybir.AxisListType.X,
                            op=mybir.AluOpType.add,
                        )
                        tmps[ci] = tmp2
                    # stats layout [c][b][s]: strided views of sums/sqs
                    sv = stats_tt[:, :nch * 16].rearrange(
                        "p (c b s) -> p c b s", b=2, s=8)
                    mean = stat.tile([P, 3, 8], F32, tag="mean", name="mean")
                    nc.vector.tensor_scalar_mul(mean[:, :nch, :],
                                                sv[:, :, 0, :], 1.0 / HD)
                    var = stat.tile([P, 3, 8], F32, tag="var", name="var")
                    nc.vector.tensor_scalar_mul(var[:, :nch, :],
                                                sv[:, :, 1, :], 1.0 / HD)
                    msq = stat.tile([P, 3, 8], F32, tag="msq", name="msq")
                    nc.vector.tensor_mul(msq[:, :nch, :], mean[:, :nch, :],
                                         mean[:, :nch, :])
                    nc.vector.tensor_sub(var[:, :nch, :], var[:, :nch, :],
                                         msq[:, :nch, :])
                    std = stat.tile([P, 3, 8], F32, tag="std", name="std")
                    nc.scalar.activation(
                        std[:, :nch, :], var[:, :nch, :],
                        mybir.ActivationFunctionType.Sqrt, bias=eps_t[:]
                    )
                    rstd = stat.tile([P, 3, 8], F32, tag="rstd", name="rstd")
                    nc.vector.reciprocal(rstd[:, :nch, :], std[:, :nch, :])
                    for i, ci in enumerate(chunks):
                        c0 = 512 * ci
                        t3 = tmps[ci][:, 0, :].rearrange("p (s h) -> p s h",
                                                         h=HD)
                        mean_c = mean[:, i, :]
                        rstd_c = rstd[:, i, :]
                        mean_b = mean_c[:, :, None].to_broadcast((P, 8, HD))
                        rstd_b = rstd_c[:, :, None].to_broadcast((P, 8, HD))
                        nc.gpsimd.tensor_tensor(t3, t3, mean_b,
                                                op=mybir.AluOpType.subtract)
                        if apply_gn:
                            nc.gpsimd.tensor_tensor(t3, t3, rstd_b,
                                                    op=mybir.AluOpType.mult)
                            gamma_b = gamma_bc[:, None, :].to_broadcast(
                                (P, 8, HD))
                            nc.gpsimd.tensor_tensor(t3, t3, gamma_b,
                                                    op=mybir.AluOpType.mult)
                        spans = []
                        if c0 < D:
                            q_hi = min(c0 + 512, D)
                            spans.append((qnp, c0, q_hi - c0, 0))
                        if c0 + 512 > D:
                            k_lo = max(c0, D)
                            spans.append((knp, k_lo - D, c0 + 512 - k_lo,
                                          k_lo - c0))
                        for (dst, d0, dlen, src_off) in spans:
                            hp0, nhp = d0 // P, dlen // P
                            sg0, nsg_s = src_off // HD, dlen // HD
                            src2 = t3[:, sg0:sg0 + nsg_s, :].rearrange(
                                "p (a s) h -> p a s h", a=nhp)
                            dgt = dst[:, hp0:hp0 + nhp, tt, :].rearrange(
                                "p a (s h) -> p a s h", h=HD)
                            if apply_gn:
                                beta_b = beta_bc[:, None, None, :].to_broadcast(
                                    (P, nhp, 2, HD))
                                nc.gpsimd.tensor_tensor(
                                    dgt, src2, beta_b, op=mybir.AluOpType.add
                                )
                            else:
                                rstd_s = rstd_c[:, sg0:sg0 + nsg_s].rearrange(
                                    "p (a s) -> p a s", a=nhp)[
                                    :, :, :, None].to_broadcast((P, nhp, 2, HD))
                                nc.gpsimd.tensor_tensor(
                                    dgt, src2, rstd_s, op=mybir.AluOpType.mult
                                )

            emit_ln_sweep([0, 1])
            emit_ln_sweep([2])
            # v chunks (no LN): bias add, cast bf16, scatter into vext
            for tt in range(NT):
                for (c0, cs) in ((1536, 512), (2048, 256)):
                    psum_full = ps_v.tile([P, 512], F32, tag="v", name="psum_v")
                    psum = psum_full[:, :cs]
                    for dc in range(DC):
                        nc.tensor.matmul(
                            psum,
                            lhsT=xT[:, dc, tt * P:(tt + 1) * P],
                            rhs=qkvwT[:, dc, c0:c0 + cs],
                            start=(dc == 0),
                            stop=False,
                        )
                    nc.tensor.matmul(
                        psum, lhsT=ones1[:], rhs=brow[:, c0:c0 + cs],
                        start=False, stop=True,
                    )
                    hs = (c0 - 2 * D) // HD
                    nh = cs // HD
                    nc.scalar.copy(
                        vext[:, tt, hs:hs + nh, 0:HD],
                        psum.rearrange("p (s h) -> p s h", h=HD),
                    )

        # ---- per-head attention ----
        # attnoutT [o_in, o_out, t] is written directly by the normalize step
        attnoutT = data.tile([P, DC, N], BF16)
        qp_sets = []
        for si in range(2):
            q0 = data.tile([P, N], BF16, tag=f"qp0_{si}", name="qp0p")
            q1 = data.tile([P, N], BF16, tag=f"qp1_{si}", name="qp1p")
            nc.vector.memset(q0[HD:2 * HD, :], 0.0)
            nc.vector.memset(q1[0:HD, :], 0.0)
            qp_sets.append((q0, q1))
        with tc.tile_pool(name="ps_st", bufs=2, space="PSUM") as ps_st, \
             tc.tile_pool(name="ps_av", bufs=4, space="PSUM") as ps_av:

            def emit_pair_transposes(hp):
                # kkT: [hd, t] packed pair -- head 2hp in partitions 0:64,
                # 2hp+1 in 64:128, transposed on the (otherwise idle) DMA
                # engines.  qp0/qp1: per-head q, zero-padded in the other
                # head's partitions, so scores run at K=128 (full PE array
                # activity keeps the HAM clock-gate warm) with the cross-head
                # products nulled by the zeros.
                kkT = qkpool.tile([P, N], BF16, tag="kkT", name="kkT")
                qqT = qkpool.tile([P, N], BF16, tag="qqT", name="qqT")
                nc.sync.dma_start_transpose(
                    kkT.rearrange("p (b t) -> p b t", t=P), knp[:, hp])
                nc.sync.dma_start_transpose(
                    qqT.rearrange("p (b t) -> p b t", t=P), qnp[:, hp])
                qp0, qp1 = qp_sets[hp % 2]
                nc.vector.tensor_copy(qp0[0:HD, :], qqT[0:HD, :])
                nc.vector.tensor_copy(qp1[HD:2 * HD, :], qqT[HD:2 * HD, :])
                return kkT, qp0, qp1

            def emit_head(h, kkT, qp0, qp1, prev):
                """Emit scores+exp for head h, 1:1 interleaved with the
                attn@v accumulation of head h-1 (prev) so the static TensorE
                stream has ready work during every exp drain."""
                qT = qp0 if h % 2 == 0 else qp1
                E = epool.tile([P, NT, N], BF16, tag="E", name="E")
                if prev is not None:
                    hprev, Eprev = prev
                    pa0 = ps_av.tile([P, 512], F32, tag="av", name="pa0")
                    pa1 = ps_av.tile([P, 512], F32, tag="av", name="pa1")
                for jt in range(NT):
                    ps = ps_st.tile([P, N], F32, tag="st", name="ps_st_t")
                    for ic in range(2):
                        nc.tensor.matmul(
                            ps[:, ic * 512:(ic + 1) * 512],
                            lhsT=kkT[:, jt * P:(jt + 1) * P],
                            rhs=qT[:, ic * 512:(ic + 1) * 512],
                            start=True,
                            stop=True,
                        )
                    nc.scalar.activation(
                        E[:, jt, :],
                        ps,
                        mybir.ActivationFunctionType.Exp,
                        scale=SCALE,
                    )
                    if prev is not None:
                        nc.tensor.matmul(
                            pa0, lhsT=vext[:, jt, hprev, :],
                            rhs=Eprev[:, jt, 0:512],
                            start=(jt == 0), stop=(jt == NT - 1),
                        )
                        nc.tensor.matmul(
                            pa1, lhsT=vext[:, jt, hprev, :],
                            rhs=Eprev[:, jt, 512:1024],
                            start=(jt == 0), stop=(jt == NT - 1),
                        )
                if prev is not None:
                    emit_normalize(hprev, pa0, pa1)
                return E

            def emit_av_tail(h, E):
                pa0 = ps_av.tile([P, 512], F32, tag="av", name="pa0")
                pa1 = ps_av.tile([P, 512], F32, tag="av", name="pa1")
                for jt in range(NT):
                    nc.tensor.matmul(
                        pa0, lhsT=vext[:, jt, h, :], rhs=E[:, jt, 0:512],
                        start=(jt == 0), stop=(jt == NT - 1),
                    )
                    nc.tensor.matmul(
                        pa1, lhsT=vext[:, jt, h, :], rhs=E[:, jt, 512:1024],
                        start=(jt == 0), stop=(jt == NT - 1),
                    )
                emit_normalize(h, pa0, pa1)

            def emit_normalize(h, pa0, pa1):
                for ic, pa in ((0, pa0), (1, pa1)):
                    s_sb = nrm.tile([HD, 512], F32, tag="s_sb", name="s_sb")
                    nc.vector.tensor_copy(s_sb[:], pa[HD:2 * HD, :])
                    rcp_t = nrm.tile([HD, 512], F32, tag="rcp_t", name="rcp_t")
                    nc.vector.reciprocal_approx_fast(rcp_t[:], s_sb[:])
                    nc.vector.tensor_tensor(
                        attnoutT[(h % 2) * HD:(h % 2 + 1) * HD, h // 2,
                                 ic * 512:(ic + 1) * 512],
                        pa[0:HD, :],
                        rcp_t[:],
                        op=mybir.AluOpType.mult,
                    )

            cur = emit_pair_transposes(0)
            prev = None  # (h, E)
            for h in range(H):
                hp, hh = divmod(h, 2)
                if hh == 0 and hp > 0:
                    cur = nxt
                E = emit_head(h, *cur, prev)
                if hh == 1 and hp + 1 < H // 2:
                    nxt = emit_pair_transposes(hp + 1)
                prev = (h, E)
            emit_av_tail(*prev)

        # ---- output projection: outT[e, t] = projwT.T @ attnoutT ----
        with tc.tile_pool(name="ps_pj", bufs=4, space="PSUM") as ps_pj:
            for et in range(DC):
                ps0 = ps_pj.tile([P, 512], F32, tag="pj", name="ps_pj0")
                ps1 = ps_pj.tile([P, 512], F32, tag="pj", name="ps_pj1")
                for oc in range(DC):
                    for tc2, ps in ((0, ps0), (1, ps1)):
                        nc.tensor.matmul(
                            ps,
                            lhsT=projwT[:, oc, et * P:(et + 1) * P],
                            rhs=attnoutT[:, oc, tc2 * 512:(tc2 + 1) * 512],
                            start=(oc == 0),
                            stop=(oc == DC - 1),
                        )
                for tc2, ps in ((0, ps0), (1, ps1)):
                    ot = outp.tile([P, 512], F32, tag="outt", name="ot")
                    nc.scalar.activation(
                        ot[:], ps, mybir.ActivationFunctionType.Identity,
                        bias=projb_col[:, et:et + 1],
                    )
                    nc.sync.dma_start(
                        out_d[et * P:(et + 1) * P, tc2 * 512:(tc2 + 1) * 512],
                        ot[:],
                    )

_NC_CACHE = {}


def _get_nc(apply_gn=True):
    if apply_gn not in _NC_CACHE:
        _NC_CACHE[apply_gn] = _build_graph(apply_gn)
    return _NC_CACHE[apply_gn]


def make_in_maps(x, qkv_w, qkv_b, proj_w, proj_b, qn_gamma, qn_beta):
    """Host-side layout prep: transpose + bf16-cast x and weight matrices so
    the kernel DMAs them straight into contraction-on-partitions layouts."""
    import ml_dtypes
    bf = ml_dtypes.bfloat16
    x = np.asarray(x, np.float32)
    shared = {
        "qkv_w": np.ascontiguousarray(np.asarray(qkv_w, np.float32).T.astype(bf)),
        "qkv_b": np.ascontiguousarray(qkv_b, np.float32),
        "qkv_br": np.ascontiguousarray(
            np.asarray(qkv_b, np.float32).reshape(1, -1).astype(bf)),
        "proj_w": np.ascontiguousarray(np.asarray(proj_w, np.float32).T.astype(bf)),
        "proj_b": np.ascontiguousarray(proj_b, np.float32),
        "qn_gamma": np.ascontiguousarray(qn_gamma, np.float32),
        "qn_beta": np.ascontiguousarray(qn_beta, np.float32),
    }
    return [
        {**shared, "x": np.ascontiguousarray(x[i].T.astype(bf))} for i in range(B)
    ]


def extract_output(res):
    return np.stack(
        [np.ascontiguousarray(res.results[i]["out"].T) for i in range(B)], axis=0
    )


def kernel(x, qkv_w, qkv_b, proj_w, proj_b, qn_gamma, qn_beta):
    qn_gamma = np.ascontiguousarray(qn_gamma, np.float32)
    qn_beta = np.ascontiguousarray(qn_beta, np.float32)
    apply_gn = not (np.all(qn_gamma == 1.0) and np.all(qn_beta == 0.0))
    nc = _get_nc(apply_gn)
    in_maps = make_in_maps(x, qkv_w, qkv_b, proj_w, proj_b, qn_gamma, qn_beta)
    res = run_bass_kernel_spmd(nc, in_maps, core_ids=list(range(B)))
    return extract_output(res)

